# revision 26
# baseline (speedup 1.0000x reference)
"""Trainium2 Bass kernel for nn_LinearGML2 (fp8 DoubleRow, layout B).

Computes out[b, k] = || (x_b - w_k) @ L_k ||_2 for K=256 classes
(L_k lower-triangular: diag = L_diags**2, strict lower = L_lower),
B=1024, d=512.  Sharded over classes: 32 per core.

Design (per core):
  * Linearity: z = x @ L_k - w_k @ L_k.  Host quantizes x, L, -w to
    fp8e4 (pure dtype/layout transforms); the subtraction happens on
    device via a per-partition bias in the epilogue.
  * Main matmuls in "layout B": out z^T[e, b] so TensorE streams the
    (class-independent!) xq as moving operand at fp8 DoubleRow rate
    (2 elem/cyc).  Stationary = L_k blocks [128f, 2plane, 128e];
    lower-triangular structure skips 6/16 of the blocks.
    Per class: 6 (eb, db) groups x (1 t-MM + 4 main MMs).
  * t_k = -w_k @ L_k computed by a tiny N=1 MM per group reusing the
    same stationary -> psum_t -> gpsimd copy to SBUF.
  * Epilogue per eb: sq = Square(z + (-t)) via one ACT pass
    (per-partition bias!) PSUM->SBUF fp8 (eb0-2), DVE 2-pass for eb3.
  * Partition-sum Σ_e sq via fp8-DR "ones" MMs: stationary is an
    offset-sliced identity strip so row 32*bq+k of a single-bank
    psum accumulator collects class k's b-segment bq.  One sqrt +
    one DMA at the end.
"""

from contextlib import ExitStack

import ml_dtypes
import numpy as np

import concourse.bass as bass  # noqa: F401
import concourse.tile as tile
from concourse import bacc, mybir
from concourse._compat import with_exitstack
from concourse.alu_op_type import AluOpType
from concourse.bass_utils import run_bass_kernel_spmd

K_CLASSES = 256
D = 512
B = 1024
N_CORES = 8
KC = K_CLASSES // N_CORES  # 32 classes per core
P = 128

F8 = ml_dtypes.float8_e4m3
_F8 = mybir.dt.float8e4
_F32 = mybir.dt.float32
_SQUARE = mybir.ActivationFunctionType.Square
_SQRT = mybir.ActivationFunctionType.Sqrt
_COPY = mybir.ActivationFunctionType.Copy
_DR = mybir.MatmulPerfMode.DoubleRow
_DRS = mybir.MatmulPerfMode.DoubleRowSwInterleave

# (eb, db) groups; L block (f in [256db, 256db+256), e in [128eb, 128eb+128))
# is nonzero iff f-range intersects [128eb, 512) (lower triangular, f >= e).
GROUPS = [(0, 0), (1, 0), (0, 1), (1, 1), (2, 1), (3, 1)]
EB_DBS = {0: (0, 1), 1: (0, 1), 2: (1,), 3: (1,)}


@with_exitstack
def _gml2_kernel(ctx: ExitStack, tc: "tile.TileContext", out, xq, lt, wqn, ek):
    nc = tc.nc
    const = ctx.enter_context(tc.tile_pool(name="const", bufs=1))
    lpool = ctx.enter_context(tc.tile_pool(name="lq", bufs=5))
    sqpool = ctx.enter_context(tc.tile_pool(name="sq", bufs=3))
    tpool = ctx.enter_context(tc.tile_pool(name="tsb", bufs=3))
    tmppool = ctx.enter_context(tc.tile_pool(name="tmp", bufs=2))
    zp = ctx.enter_context(tc.tile_pool(name="zp", bufs=3, space="PSUM"))
    ptp = ctx.enter_context(tc.tile_pool(name="ptp", bufs=1, space="PSUM"))
    pfp = ctx.enter_context(tc.tile_pool(name="pfp", bufs=1, space="PSUM"))

    xq_sb = const.tile([P, 2, 2, 1024], _F8, name="xq_sb")  # (db, plane, b)
    wq_sb = const.tile([P, 2, 2, KC], _F8, name="wq_sb")  # (db, plane, k); holds -w
    ek_sb = const.tile([P, 2, 256], _F8, name="ek_sb")  # identity strip
    outsb = const.tile([P, 256], _F32, name="outsb")
    # first MMs need wq + xq[db0] (bq0 first) + lt[0]; order DMAs so those
    # deps land first, split across queues
    nc.gpsimd.dma_start(wq_sb[:, :, :, :], wqn)
    nc.gpsimd.dma_start(xq_sb[:, 0, :, 0:256], xq[0][:, :, 0:256])
    nc.gpsimd.dma_start(xq_sb[:, 0, :, 256:1024], xq[0][:, :, 256:1024])
    nc.scalar.dma_start(xq_sb[:, 1, :, 0:512], xq[1][:, :, 0:512])
    nc.scalar.dma_start(xq_sb[:, 1, :, 512:1024], xq[1][:, :, 512:1024])
    nc.scalar.dma_start(ek_sb[:, :, :], ek)

    pt = ptp.tile([P, 512], _F32, name="pt")  # cols 0:8 used (parity half, eb)
    pf = pfp.tile([P, 512], _F32, name="pf")  # cols 0:256: row 32*bq+k


    lts = {}

    def fetch(k):
        if k < KC:
            t = lpool.tile([P, 6, 2, P], _F8, tag="lt", name="lt")
            nc.sync.dma_start(t[:, :, :, :], lt[k])
            lts[k] = t

    def emit_ones(j, sq_j, start, stop, ebp_order=(0, 1)):
        for bq in range(4):
            v = 32 * bq + j
            lwv = ek_sb[:, :, 127 - v : 255 - v]
            for ei, ebp in enumerate(ebp_order):
                nc.tensor.matmul(
                    pf[:, 0:256],
                    lwv,
                    sq_j[:, 2 * ebp : 2 * ebp + 2, bq * 256 : bq * 256 + 256],
                    start=(start and bq == 0 and ei == 0),
                    stop=(stop and bq == 3 and ei == 1),
                    perf_mode=_DR,
                )

    fetch(0)
    fetch(1)
    fetch(2)
    sqs = {}
    for k in range(KC):
        lw_all = lts.pop(k)
        sq_k = sqpool.tile([P, 4, 1024], _F8, tag="sq", name="sq")
        t_sb = tpool.tile([P, 4], _F32, tag="tsb", name="tsb")
        h = (k % 2) * 4
        zts = {}
        groups_k = GROUPS if k < KC - 1 else [GROUPS[i] for i in (5, 4, 0, 1, 2, 3)]
        gidx = list(range(6)) if k < KC - 1 else [5, 4, 0, 1, 2, 3]
        for gi, (eb, db) in enumerate(groups_k):
            g = gidx[gi]
            lw = lw_all[:, g, :, :]
            first = db == EB_DBS[eb][0]
            last = db == EB_DBS[eb][-1]
            nc.tensor.matmul(
                pt[:, h + eb : h + eb + 1],
                lw,
                wq_sb[:, db, :, k : k + 1],
                start=(gi == 0),
                stop=last,
                perf_mode=_DRS,
                skip_group_check=True,
            )
            if first:
                zts[eb] = zp.tile([P, 1024], _F32, tag="zt", name="zt")
            zt = zts[eb]
            for bq in range(4):
                nc.tensor.matmul(
                    zt[:, bq * 256 : bq * 256 + 256],
                    lw,
                    xq_sb[:, db, :, bq * 256 : bq * 256 + 256],
                    start=(first and bq % 2 == 0),
                    stop=last,
                    perf_mode=_DRS,
                    skip_group_check=True,
                )
            if last:
                # t for this eb final: stage to SBUF on DVE (ACT is the
                # drain bottleneck), then drain z -> sq with the subtract
                # fused as the activation bias.
                nc.vector.tensor_copy(t_sb[:, eb : eb + 1], pt[:, h + eb : h + eb + 1])
                if eb < 3:
                    nc.scalar.activation(
                        sq_k[:, eb, :], zt[:, :], _SQUARE,
                        bias=t_sb[:, eb : eb + 1], scale=1.0,
                    )
                else:
                    tmp = tmppool.tile([P, 1024], _F32, tag="tmp", name="tmp")
                    nc.vector.tensor_scalar_add(tmp[:, :], zt[:, :], t_sb[:, eb : eb + 1])
                    nc.vector.tensor_tensor(
                        sq_k[:, eb, :], tmp[:, :], tmp[:, :], AluOpType.mult
                    )
        sqs[k] = sq_k
        fetch(k + 3)
        if k >= 1:
            emit_ones(k - 1, sqs.pop(k - 1), start=(k - 1 == 0), stop=False)
    # last class drains eb2/eb3 first (group reorder), so emit its
    # ebp1 ones-MMs first to overlap the eb0/eb1 drains
    emit_ones(KC - 1, sqs.pop(KC - 1), start=False, stop=True, ebp_order=(1, 0))
    nc.scalar.activation(outsb[:, :], pf[:, 0:256], _SQRT)
    nc.sync.dma_start(out, outsb[:, :])


_CACHE: dict = {}


def build_nc():
    if "nc" in _CACHE:
        return _CACHE["nc"]
    nc = bacc.Bacc("TRN2", target_bir_lowering=False, debug=False, num_devices=N_CORES)
    xq = nc.dram_tensor("xq", [2, P, 2, B], _F8, kind="ExternalInput").ap()
    lt = nc.dram_tensor("lt", [KC, P, 6, 2, P], _F8, kind="ExternalInput").ap()
    wqn = nc.dram_tensor("wqn", [P, 2, 2, KC], _F8, kind="ExternalInput").ap()
    ek = nc.dram_tensor("ek", [P, 2, 256], _F8, kind="ExternalInput").ap()
    out = nc.dram_tensor("out", [P, 256], _F32, kind="ExternalOutput").ap()
    with tile.TileContext(nc) as tc:
        _gml2_kernel(tc, out, xq, lt, wqn, ek)
    nc.compile()
    _CACHE["nc"] = nc
    return nc


def host_prep(inputs, weight, L_diags, L_lower):
    """Layout/dtype transforms only (beyond L assembly, same as reference)."""
    x = np.asarray(inputs, dtype=np.float32)
    w = np.asarray(weight, dtype=np.float32).reshape(K_CLASSES, D)
    ld = np.asarray(L_diags, dtype=np.float32)
    ll = np.asarray(L_lower, dtype=np.float32)

    lmat = np.zeros((K_CLASSES, D, D), dtype=np.float32)
    ri, ci = np.tril_indices(D, k=-1)
    lmat[:, ri, ci] = ll
    dd = np.arange(D)
    lmat[:, dd, dd] = ld * ld

    # xq[db, p, i, b] = fp8(x[b, 256*db + 128*i + p])
    xq = np.ascontiguousarray(
        x.T.reshape(2, 2, P, B).transpose(0, 2, 1, 3)
    ).astype(F8)
    # lt[k, p, g, i, m] = fp8(L[k, 256*db+128*i+p, 128*eb+m]) for g=(eb,db)
    l8 = lmat.astype(F8)  # [K, 512, 512]
    lf = l8.reshape(K_CLASSES, 2, 2, P, 4, P)  # [k, db, i, p, eb, m]
    lt = np.empty((K_CLASSES, P, 6, 2, P), dtype=F8)
    for g, (eb, db) in enumerate(GROUPS):
        lt[:, :, g, :, :] = lf[:, db, :, :, eb, :].transpose(0, 2, 1, 3)
    # DoubleRowSwInterleave weight layout: stream pos j = 2*(127-m) + i
    jj = np.arange(2 * P)
    mm_idx = P - 1 - jj // 2
    ii_idx = jj % 2
    lt = np.ascontiguousarray(
        lt.reshape(K_CLASSES, P, 6, 2 * P)[..., ii_idx * P + mm_idx]
    ).reshape(K_CLASSES, P, 6, 2, P)
    # wqn[p, db, i, k] = fp8(-w[k, 256*db + 128*i + p])
    wqn_all = np.ascontiguousarray(
        (-w.astype(F8).astype(np.float32)).astype(F8).T.reshape(2, 2, P, K_CLASSES).transpose(2, 0, 1, 3)
    ).astype(F8)
    # identity strip: [p, i, j] = 1 iff j == 127
    ek = np.zeros((P, 2, 256), dtype=F8)
    ek[:, :, 127] = 1.0
    return xq, lt, wqn_all, ek


def make_in_maps(xq, lt, wqn_all, ek):
    in_maps = []
    for c in range(N_CORES):
        sl = slice(c * KC, (c + 1) * KC)
        in_maps.append(
            {
                "xq": xq,
                "lt": np.ascontiguousarray(lt[sl]),
                "wqn": np.ascontiguousarray(wqn_all[:, :, :, sl]),
                "ek": ek,
            }
        )
    return in_maps


def kernel(inputs, weight, L_diags, L_lower, **run_kwargs):
    xq, lt, wqn_all, ek = host_prep(inputs, weight, L_diags, L_lower)
    nc = build_nc()
    in_maps = make_in_maps(xq, lt, wqn_all, ek)
    res = run_bass_kernel_spmd(nc, in_maps, core_ids=list(range(N_CORES)), **run_kwargs)
    out = np.empty((B, K_CLASSES), dtype=np.float32)
    for c in range(N_CORES):
        o = np.asarray(res.results[c]["out"]).astype(np.float32)
        o = o.reshape(4, KC, 256)  # [bq, k, b_within]
        out[:, c * KC : (c + 1) * KC] = o.transpose(0, 2, 1).reshape(B, KC)
    if run_kwargs:
        _CACHE["last_result"] = res
    return out


# revision 27
# speedup vs baseline: 1.0277x; 1.0277x over previous
"""Trainium2 Bass kernel for nn_LinearGML2 (fp8 DoubleRow, layout B).

Computes out[b, k] = || (x_b - w_k) @ L_k ||_2 for K=256 classes
(L_k lower-triangular: diag = L_diags**2, strict lower = L_lower),
B=1024, d=512.  Sharded over classes: 32 per core.

Design (per core):
  * Linearity: z = x @ L_k - w_k @ L_k.  Host quantizes x, L, -w to
    fp8e4 (pure dtype/layout transforms); the subtraction happens on
    device via a per-partition bias in the epilogue.
  * Main matmuls in "layout B": out z^T[e, b] so TensorE streams the
    (class-independent!) xq as moving operand at fp8 DoubleRow rate
    (2 elem/cyc).  Stationary = L_k blocks [128f, 2plane, 128e];
    lower-triangular structure skips 6/16 of the blocks.
    Per class: 6 (eb, db) groups x (1 t-MM + 4 main MMs).
  * t_k = -w_k @ L_k computed by a tiny N=1 MM per group reusing the
    same stationary -> psum_t -> gpsimd copy to SBUF.
  * Epilogue per eb: sq = Square(z + (-t)) via one ACT pass
    (per-partition bias!) PSUM->SBUF fp8 (eb0-2), DVE 2-pass for eb3.
  * Partition-sum Σ_e sq via fp8-DR "ones" MMs: stationary is an
    offset-sliced identity strip so row 32*bq+k of a single-bank
    psum accumulator collects class k's b-segment bq.  One sqrt +
    one DMA at the end.
"""

from contextlib import ExitStack

import ml_dtypes
import numpy as np

import concourse.bass as bass  # noqa: F401
import concourse.tile as tile
from concourse import bacc, mybir
from concourse._compat import with_exitstack
from concourse.alu_op_type import AluOpType
from concourse.bass_utils import run_bass_kernel_spmd

K_CLASSES = 256
D = 512
B = 1024
N_CORES = 8
KC = K_CLASSES // N_CORES  # 32 classes per core
P = 128

F8 = ml_dtypes.float8_e4m3
_F8 = mybir.dt.float8e4
_F32 = mybir.dt.float32
_SQUARE = mybir.ActivationFunctionType.Square
_SQRT = mybir.ActivationFunctionType.Sqrt
_COPY = mybir.ActivationFunctionType.Copy
_DR = mybir.MatmulPerfMode.DoubleRow
_DRS = mybir.MatmulPerfMode.DoubleRowSwInterleave

# (eb, db) groups; L block (f in [256db, 256db+256), e in [128eb, 128eb+128))
# is nonzero iff f-range intersects [128eb, 512) (lower triangular, f >= e).
GROUPS = [(0, 0), (1, 0), (0, 1), (1, 1), (2, 1), (3, 1)]
EB_DBS = {0: (0, 1), 1: (0, 1), 2: (1,), 3: (1,)}


@with_exitstack
def _gml2_kernel(ctx: ExitStack, tc: "tile.TileContext", out, xq, lt, wqn, ek):
    nc = tc.nc
    const = ctx.enter_context(tc.tile_pool(name="const", bufs=1))
    lpool = ctx.enter_context(tc.tile_pool(name="lq", bufs=4))
    sqpool = ctx.enter_context(tc.tile_pool(name="sq", bufs=3))
    tpool = ctx.enter_context(tc.tile_pool(name="tsb", bufs=2))
    tmppool = ctx.enter_context(tc.tile_pool(name="tmp", bufs=2))
    zp = ctx.enter_context(tc.tile_pool(name="zp", bufs=3, space="PSUM"))
    ptp = ctx.enter_context(tc.tile_pool(name="ptp", bufs=1, space="PSUM"))
    pfp = ctx.enter_context(tc.tile_pool(name="pfp", bufs=1, space="PSUM"))

    xq_sb = const.tile([P, 2, 2, 1024], _F8, name="xq_sb")  # (db, plane, b)
    wq_sb = const.tile([P, 2, 2, KC], _F8, name="wq_sb")  # (db, plane, k); holds -w
    ek_sb = const.tile([P, 2, 256], _F8, name="ek_sb")  # identity strip
    outsb = const.tile([P, 256], _F32, name="outsb")
    # first MMs need wq + xq[db0] (bq0 first) + lt[0]; order DMAs so those
    # deps land first, split across queues
    nc.gpsimd.dma_start(wq_sb[:, :, :, :], wqn)
    nc.gpsimd.dma_start(xq_sb[:, 0, :, 0:256], xq[0][:, :, 0:256])
    nc.gpsimd.dma_start(xq_sb[:, 0, :, 256:1024], xq[0][:, :, 256:1024])
    nc.scalar.dma_start(xq_sb[:, 1, :, 0:512], xq[1][:, :, 0:512])
    nc.scalar.dma_start(xq_sb[:, 1, :, 512:1024], xq[1][:, :, 512:1024])
    nc.scalar.dma_start(ek_sb[:, :, :], ek)

    pt = ptp.tile([P, 512], _F32, name="pt")  # cols 0:8 used (parity half, eb)
    pf = pfp.tile([P, 512], _F32, name="pf")  # cols 0:256: row 32*bq+k


    lts = {}

    def fetch(k):
        if k < KC:
            t = lpool.tile([P, 6, 2, P], _F8, tag="lt", name="lt")
            nc.sync.dma_start(t[:, :, :, :], lt[k])
            lts[k] = t

    def emit_ones(j, sq_j, start, stop, ebp_order=(0, 1)):
        for bq in range(4):
            v = 32 * bq + j
            lwv = ek_sb[:, :, 127 - v : 255 - v]
            for ei, ebp in enumerate(ebp_order):
                nc.tensor.matmul(
                    pf[:, 0:256],
                    lwv,
                    sq_j[:, 2 * ebp : 2 * ebp + 2, bq * 256 : bq * 256 + 256],
                    start=(start and bq == 0 and ei == 0),
                    stop=(stop and bq == 3 and ei == 1),
                    perf_mode=_DR,
                )

    fetch(0)
    fetch(1)
    fetch(2)
    sqs = {}
    for k in range(KC):
        lw_all = lts.pop(k)
        sq_k = sqpool.tile([P, 4, 1024], _F8, tag="sq", name="sq")
        t_sb = tpool.tile([P, 4], _F32, tag="tsb", name="tsb")
        h = (k % 2) * 4
        zts = {}
        groups_k = GROUPS if k < KC - 1 else [GROUPS[i] for i in (5, 4, 0, 1, 2, 3)]
        gidx = list(range(6)) if k < KC - 1 else [5, 4, 0, 1, 2, 3]
        for gi, (eb, db) in enumerate(groups_k):
            g = gidx[gi]
            lw = lw_all[:, g, :, :]
            first = db == EB_DBS[eb][0]
            last = db == EB_DBS[eb][-1]
            nc.tensor.matmul(
                pt[:, h + eb : h + eb + 1],
                lw,
                wq_sb[:, db, :, k : k + 1],
                start=(gi == 0),
                stop=last,
                perf_mode=_DRS,
                skip_group_check=True,
            )
            if first:
                zts[eb] = zp.tile([P, 1024], _F32, tag="zt", name="zt")
            zt = zts[eb]
            for bq in range(4):
                nc.tensor.matmul(
                    zt[:, bq * 256 : bq * 256 + 256],
                    lw,
                    xq_sb[:, db, :, bq * 256 : bq * 256 + 256],
                    start=(first and bq % 2 == 0),
                    stop=last,
                    perf_mode=_DRS,
                    skip_group_check=True,
                )
            if last:
                # t for this eb final: stage to SBUF on DVE (ACT is the
                # drain bottleneck), then drain z -> sq with the subtract
                # fused as the activation bias.
                nc.vector.tensor_copy(t_sb[:, eb : eb + 1], pt[:, h + eb : h + eb + 1])
                if eb < 3:
                    nc.scalar.activation(
                        sq_k[:, eb, :], zt[:, :], _SQUARE,
                        bias=t_sb[:, eb : eb + 1], scale=1.0,
                    )
                else:
                    tmp = tmppool.tile([P, 1024], _F32, tag="tmp", name="tmp")
                    nc.vector.tensor_scalar_add(tmp[:, :], zt[:, :], t_sb[:, eb : eb + 1])
                    nc.vector.tensor_tensor(
                        sq_k[:, eb, :], tmp[:, :], tmp[:, :], AluOpType.mult
                    )
        sqs[k] = sq_k
        fetch(k + 3)
        if k >= 1:
            emit_ones(k - 1, sqs.pop(k - 1), start=(k - 1 == 0), stop=False)
    # last class drains eb2/eb3 first (group reorder), so emit its
    # ebp1 ones-MMs first to overlap the eb0/eb1 drains
    emit_ones(KC - 1, sqs.pop(KC - 1), start=False, stop=True, ebp_order=(1, 0))
    nc.scalar.activation(outsb[:, :], pf[:, 0:256], _SQRT)
    nc.sync.dma_start(out, outsb[:, :])


_CACHE: dict = {}


def build_nc():
    if "nc" in _CACHE:
        return _CACHE["nc"]
    nc = bacc.Bacc("TRN2", target_bir_lowering=False, debug=False, num_devices=N_CORES)
    xq = nc.dram_tensor("xq", [2, P, 2, B], _F8, kind="ExternalInput").ap()
    lt = nc.dram_tensor("lt", [KC, P, 6, 2, P], _F8, kind="ExternalInput").ap()
    wqn = nc.dram_tensor("wqn", [P, 2, 2, KC], _F8, kind="ExternalInput").ap()
    ek = nc.dram_tensor("ek", [P, 2, 256], _F8, kind="ExternalInput").ap()
    out = nc.dram_tensor("out", [P, 256], _F32, kind="ExternalOutput").ap()
    with tile.TileContext(nc) as tc:
        _gml2_kernel(tc, out, xq, lt, wqn, ek)
    nc.compile()
    _CACHE["nc"] = nc
    return nc


def host_prep(inputs, weight, L_diags, L_lower):
    """Layout/dtype transforms only (beyond L assembly, same as reference)."""
    x = np.asarray(inputs, dtype=np.float32)
    w = np.asarray(weight, dtype=np.float32).reshape(K_CLASSES, D)
    ld = np.asarray(L_diags, dtype=np.float32)
    ll = np.asarray(L_lower, dtype=np.float32)

    lmat = np.zeros((K_CLASSES, D, D), dtype=np.float32)
    ri, ci = np.tril_indices(D, k=-1)
    lmat[:, ri, ci] = ll
    dd = np.arange(D)
    lmat[:, dd, dd] = ld * ld

    # xq[db, p, i, b] = fp8(x[b, 256*db + 128*i + p])
    xq = np.ascontiguousarray(
        x.T.reshape(2, 2, P, B).transpose(0, 2, 1, 3)
    ).astype(F8)
    # lt[k, p, g, i, m] = fp8(L[k, 256*db+128*i+p, 128*eb+m]) for g=(eb,db)
    l8 = lmat.astype(F8)  # [K, 512, 512]
    lf = l8.reshape(K_CLASSES, 2, 2, P, 4, P)  # [k, db, i, p, eb, m]
    lt = np.empty((K_CLASSES, P, 6, 2, P), dtype=F8)
    for g, (eb, db) in enumerate(GROUPS):
        lt[:, :, g, :, :] = lf[:, db, :, :, eb, :].transpose(0, 2, 1, 3)
    # DoubleRowSwInterleave weight layout: stream pos j = 2*(127-m) + i
    jj = np.arange(2 * P)
    mm_idx = P - 1 - jj // 2
    ii_idx = jj % 2
    lt = np.ascontiguousarray(
        lt.reshape(K_CLASSES, P, 6, 2 * P)[..., ii_idx * P + mm_idx]
    ).reshape(K_CLASSES, P, 6, 2, P)
    # wqn[p, db, i, k] = fp8(-w[k, 256*db + 128*i + p])
    wqn_all = np.ascontiguousarray(
        (-w.astype(F8).astype(np.float32)).astype(F8).T.reshape(2, 2, P, K_CLASSES).transpose(2, 0, 1, 3)
    ).astype(F8)
    # identity strip: [p, i, j] = 1 iff j == 127
    ek = np.zeros((P, 2, 256), dtype=F8)
    ek[:, :, 127] = 1.0
    return xq, lt, wqn_all, ek


def make_in_maps(xq, lt, wqn_all, ek):
    in_maps = []
    for c in range(N_CORES):
        sl = slice(c * KC, (c + 1) * KC)
        in_maps.append(
            {
                "xq": xq,
                "lt": np.ascontiguousarray(lt[sl]),
                "wqn": np.ascontiguousarray(wqn_all[:, :, :, sl]),
                "ek": ek,
            }
        )
    return in_maps


def kernel(inputs, weight, L_diags, L_lower, **run_kwargs):
    xq, lt, wqn_all, ek = host_prep(inputs, weight, L_diags, L_lower)
    nc = build_nc()
    in_maps = make_in_maps(xq, lt, wqn_all, ek)
    res = run_bass_kernel_spmd(nc, in_maps, core_ids=list(range(N_CORES)), **run_kwargs)
    out = np.empty((B, K_CLASSES), dtype=np.float32)
    for c in range(N_CORES):
        o = np.asarray(res.results[c]["out"]).astype(np.float32)
        o = o.reshape(4, KC, 256)  # [bq, k, b_within]
        out[:, c * KC : (c + 1) * KC] = o.transpose(0, 2, 1).reshape(B, KC)
    if run_kwargs:
        _CACHE["last_result"] = res
    return out
